# revision 4
# baseline (speedup 1.0000x reference)
"""Multi-head attention with KV cache on 8 Trainium2 NeuronCores.

Sharding (per the hint): data-parallel over the batch (2 groups of 4
cores), tensor-parallel over the 16 heads (4 heads per core).  Attention
is fully local per head; the output projection is column-split 4-ways
within each batch group after an AllGather of the per-head contexts.

Layout strategy: everything flows through the PE in "transposed"
orientation so no on-device transposes are needed:
  qkT[e, s] = (w_qk.T @ x.T chunks)      (lhsT = w chunks, rhs = xT chunks)
  scoresT[t, s] = kT-chunk.T @ qT        (softmax free-dim = s)
  ctxT'[d+1, s] = [v | 1].T-chunks @ expT  (row 64 = softmax denominator)
  outT[oc, s] = wproj-chunk.T @ mergedT
v is additionally produced in natural [s, d] orientation (separate
matmul tiling of the same projection) because present-v and the ctx
matmul's stationary operand both want it that way.

All matmuls run in float32r (TF32-like, full PE rate, ~1.5e-4 scaled
error); exp runs on the ACT engine reading PSUM directly with the 1/8
scale folded in.  The causal mask is applied structurally: fully-masked
t-chunks are skipped, the diagonal 128x128 block is applied as one
multiplicative tri pattern.  The host verifies the mask actually has
this structure and falls back to a general multiplicative-mask path (or
numpy for degenerate masks) otherwise.
"""
import os
import sys

sys.path.insert(0, "/opt/trn_rl_repo")

import numpy as np

B, S, D, H, P = 2, 2048, 1024, 16, 2048
T = P + S
DEPTH = D // H          # 64
N_CORES = 8
GROUPS = [[0, 1, 2, 3], [4, 5, 6, 7]]
NH = H // 4             # 4 local heads per core
EQK = 2 * NH * DEPTH    # 512 (q block 256 | k block 256)
EV = NH * DEPTH         # 256
OC = D // 4             # 256 output columns per core
SN = 1024               # attention s-tile width
NTCH = T // 128         # 32 t-chunks
KCH = D // 128          # 8 contraction chunks

_prog_cache = {}
LAST_RESULT = None


def _build_plan(mask2d, tri_expect):
    """Per (sn, t-chunk, half) classification of the mask.

    Returns (plan, mode) where plan[sn] is a list of
    (t0, [half-state, half-state]) with half-state one of
    ('skip',), ('full',), ('tri', off), ('dram',).
    mode is 'causal' if no 'dram' states, else 'general'.
    """
    plan = []
    mode = "causal"
    for sn in range(S // SN):
        s0 = sn * SN
        chunks = []
        for t0 in range(0, T, 128):
            halves = []
            for h5 in range(2):
                sh0 = s0 + h5 * 512
                sub = mask2d[sh0:sh0 + 512, t0:t0 + 128]
                if not sub.any():
                    halves.append(("full",))
                elif sub.all():
                    halves.append(("skip",))
                else:
                    # try the aligned-causal shape: s-cols [0, off) fully
                    # masked, [off, off+128) == tri pattern, rest valid
                    st = None
                    subT = sub.T  # [t 128, s 512]
                    for off in range(0, 512, 128):
                        cand = np.ones((128, 512), dtype=np.float32)
                        cand[:, :off] = 0.0
                        cand[:, off:off + 128] = tri_expect
                        if np.array_equal(1.0 - subT, cand):
                            st = ("tri", off)
                            break
                    if st is None:
                        st = ("dram",)
                        mode = "general"
                    halves.append(st)
            if halves[0][0] == "skip" and halves[1][0] == "skip":
                continue
            chunks.append((t0, halves))
        plan.append(chunks)
    return plan, mode


def _plan_key(plan, mode, has_bias):
    return (mode, has_bias,
            tuple((sn_i, t0, tuple(h for h in halves))
                  for sn_i, chunks in enumerate(plan)
                  for t0, halves in chunks))


def _build_program(plan, mode, has_bias):
    import concourse.bacc as bacc
    import concourse.mybir as mybir
    from concourse import tile as tile_mod

    f32 = mybir.dt.float32
    f32r = mybir.dt.float32r
    AF = mybir.ActivationFunctionType

    nc = bacc.Bacc(None)

    # ---- per-core external tensors -------------------------------------
    xbT_d = nc.dram_tensor("xbT", [D, S], f32, kind="ExternalInput")
    wqk_d = nc.dram_tensor("wqk", [D, EQK], f32, kind="ExternalInput")
    wv_d = nc.dram_tensor("wv", [D, EV], f32, kind="ExternalInput")
    pastKT_d = nc.dram_tensor("pastKT", [NH, DEPTH, P], f32, kind="ExternalInput")
    pastV_d = nc.dram_tensor("pastV", [NH, P, DEPTH], f32, kind="ExternalInput")
    tri_d = nc.dram_tensor("tri", [128, 128], f32, kind="ExternalInput")
    wproj_d = nc.dram_tensor("wproj", [D, OC], f32, kind="ExternalInput")
    if has_bias:
        bqk_d = nc.dram_tensor("bqk", [EQK], f32, kind="ExternalInput")
        bv_d = nc.dram_tensor("bv", [EV], f32, kind="ExternalInput")
        bproj_d = nc.dram_tensor("bproj", [OC], f32, kind="ExternalInput")
    if mode == "general":
        multT_d = nc.dram_tensor("multT", [T, S], f32, kind="ExternalInput")

    pkT_d = nc.dram_tensor("pkT", [NH, DEPTH, S], f32, kind="ExternalOutput")
    pv_d = nc.dram_tensor("pv", [NH, S, DEPTH], f32, kind="ExternalOutput")
    outT_d = nc.dram_tensor("outT", [OC, S], f32, kind="ExternalOutput")

    with tile_mod.TileContext(nc) as tc:
        with (
            tc.tile_pool(name="sb", bufs=1) as sb,
            tc.tile_pool(name="ps", bufs=1, space="PSUM") as ps,
            tc.tile_pool(name="dram", bufs=1, space="DRAM") as dram,
        ):
            qdram = dram.tile([2, 128, S], f32, name="qdram")   # q heads 2j,2j+1 stacked
            cc_in = dram.tile([EV, S], f32, name="cc_in")
            cc_out = dram.tile([D, S], f32, name="cc_out")

            # ---- small constants ----------------------------------------
            tri_s = sb.tile([128, 128], f32, name="tri_s")
            nc.sync.dma_start(out=tri_s[:], in_=tri_d[:])
            tri_r = sb.tile([128, 128], f32r, name="tri_r")
            nc.vector.tensor_copy(tri_r[:], tri_s[:])
            onec = sb.tile([128, 1], f32, name="onec")
            nc.gpsimd.memset(onec[:], 1.0)

            if has_bias:
                bqk_s = sb.tile([128, EQK // 128], f32, name="bqk_s")
                nc.sync.dma_start(out=bqk_s[:],
                                  in_=bqk_d.rearrange("(t p) -> p t", p=128))
                bpr_s = sb.tile([128, OC // 128], f32, name="bpr_s")
                nc.sync.dma_start(out=bpr_s[:],
                                  in_=bproj_d.rearrange("(t p) -> p t", p=128))
                bv_row = sb.tile([1, EV], f32, name="bv_row")
                nc.sync.dma_start(out=bv_row[:], in_=bv_d[None, :])
                bv_b = sb.tile([128, EV], f32, name="bv_b")
                nc.gpsimd.partition_broadcast(bv_b[:], bv_row[:])

            # ---- weights: load + round ----------------------------------
            wqkr = []
            for k in range(KCH):
                st = sb.tile([128, EQK], f32, name=f"wqkst{k}", tag="wst", bufs=2)
                nc.sync.dma_start(out=st[:], in_=wqk_d[k * 128:(k + 1) * 128, :])
                wr = sb.tile([128, EQK], f32r, name=f"wqkr{k}", tag="wqk", bufs=KCH)
                nc.vector.tensor_copy(wr[:], st[:])
                wqkr.append(wr)
            wvr = []
            for k in range(KCH):
                st = sb.tile([128, EV], f32, name=f"wvst{k}", tag="wst", bufs=2)
                nc.sync.dma_start(out=st[:], in_=wv_d[k * 128:(k + 1) * 128, :])
                wr = sb.tile([128, EV], f32r, name=f"wvr{k}", tag="wv", bufs=KCH)
                nc.vector.tensor_copy(wr[:], st[:])
                wvr.append(wr)

            # ---- phase 1: QKV projections -------------------------------
            # qkT: psum [e:128, s:512] tiles; e-tile 0,1 = q heads (0,1),(2,3);
            # e-tile 2,3 = k heads (0,1),(2,3)
            for sq in range(4):         # s quarters of 512
                xbr = []
                for k in range(KCH):
                    st = sb.tile([128, 512], f32, name=f"xbst{sq}_{k}", tag="xbst", bufs=3)
                    nc.sync.dma_start(
                        out=st[:], in_=xbT_d[k * 128:(k + 1) * 128,
                                             sq * 512:(sq + 1) * 512])
                    xr = sb.tile([128, 512], f32r, name=f"xbr{sq}_{k}",
                                 tag="xb", bufs=KCH + 1)
                    nc.vector.tensor_copy(xr[:], st[:])
                    xbr.append(xr)
                for e in range(4):
                    pq = ps.tile([128, 512], f32, name=f"pqk{sq}_{e}", tag="psA", bufs=2, padded_shape=[128, 1024])
                    for k in range(KCH):
                        nc.tensor.matmul(pq[:], wqkr[k][:, e * 128:(e + 1) * 128],
                                         xbr[k][:], start=(k == 0), stop=(k == KCH - 1))
                    qs = sb.tile([128, 512], f32, name=f"qkst{sq}_{e}",
                                 tag="qkst", bufs=2)
                    if has_bias:
                        nc.vector.tensor_scalar_add(qs[:], pq[:],
                                                    bqk_s[:, e:e + 1])
                    else:
                        nc.vector.tensor_copy(qs[:], pq[:])
                    if e < 2:
                        nc.sync.dma_start(
                            out=qdram[e, :, sq * 512:(sq + 1) * 512], in_=qs[:])
                    else:
                        j = e - 2
                        nc.sync.dma_start(
                            out=pkT_d[2 * j, :, sq * 512:(sq + 1) * 512],
                            in_=qs[0:64, :])
                        nc.sync.dma_start(
                            out=pkT_d[2 * j + 1, :, sq * 512:(sq + 1) * 512],
                            in_=qs[64:128, :])
                # v: natural orientation [s:128, e_v:256]
                for sc in range(4):
                    abs_c = sq * 4 + sc            # 128-chunk index 0..15
                    pv_ = ps.tile([128, EV], f32, name=f"pv{abs_c}", tag="psB", bufs=2, padded_shape=[128, 1024])
                    for k in range(KCH):
                        nc.tensor.matmul(pv_[:], xbr[k][:, sc * 128:(sc + 1) * 128],
                                         wvr[k][:], start=(k == 0), stop=(k == KCH - 1))
                    vt = sb.tile([128, EV], f32, name=f"vt{abs_c}", tag="vt", bufs=3)
                    if has_bias:
                        nc.vector.tensor_add(out=vt[:], in0=pv_[:], in1=bv_b[:])
                    else:
                        nc.vector.tensor_copy(vt[:], pv_[:])
                    for h in range(NH):
                        nc.sync.dma_start(
                            out=pv_d[h, abs_c * 128:(abs_c + 1) * 128, :],
                            in_=vt[:, h * 64:(h + 1) * 64])

            # ---- phase 2: attention per local head ----------------------
            for h in range(NH):
                j, r0 = h // 2, (h % 2) * 64
                # kT [64, T] f32r: past | new
                kst = sb.tile([64, P], f32, name=f"kst{h}", tag="k64st", bufs=1)
                nc.sync.dma_start(out=kst[:], in_=pastKT_d[h])
                kbuf = sb.tile([64, T], f32r, name=f"kbuf{h}", tag="kbuf", bufs=2)
                nc.vector.tensor_copy(kbuf[:, 0:P], kst[:])
                kst2 = sb.tile([64, S], f32, name=f"kst2{h}", tag="k64st", bufs=1)
                nc.sync.dma_start(out=kst2[:], in_=pkT_d[h])
                nc.vector.tensor_copy(kbuf[:, P:T], kst2[:])
                # qT [64, S] f32r
                qst = sb.tile([64, S], f32, name=f"qst{h}", tag="k64st", bufs=1)
                nc.sync.dma_start(out=qst[:], in_=qdram[j, r0:r0 + 64, :])
                qbuf = sb.tile([64, S], f32r, name=f"qbuf{h}", tag="qbuf", bufs=2)
                nc.vector.tensor_copy(qbuf[:], qst[:])
                # v [t-chunks, 65] f32r with ones column
                vbuf = sb.tile([128, NTCH * 65], f32r, name=f"vbuf{h}",
                               tag="vbuf", bufs=2)
                vbv = vbuf.rearrange("p (c e) -> p c e", e=65)
                vst = sb.tile([128, P // 128 * 64], f32, name=f"vstp{h}",
                              tag="vst", bufs=2)
                nc.sync.dma_start(out=vst.rearrange("p (c d) -> p c d", d=64),
                                  in_=pastV_d[h].rearrange("(c p) d -> p c d", p=128))
                nc.vector.tensor_copy(vbv[:, 0:P // 128, 0:64],
                                      vst.rearrange("p (c d) -> p c d", d=64))
                vst2 = sb.tile([128, S // 128 * 64], f32, name=f"vstn{h}",
                               tag="vst", bufs=2)
                nc.sync.dma_start(out=vst2.rearrange("p (c d) -> p c d", d=64),
                                  in_=pv_d[h].rearrange("(c p) d -> p c d", p=128))
                nc.vector.tensor_copy(vbv[:, P // 128:NTCH, 0:64],
                                      vst2.rearrange("p (c d) -> p c d", d=64))
                nc.vector.tensor_copy(vbv[:, :, 64:65],
                                      onec.broadcast_to([128, NTCH, 1]))

                for sn in range(S // SN):
                    s0 = sn * SN
                    chunks = plan[sn]
                    # per half: first/last active chunk index for start/stop
                    act = [[i for i, (_, hv) in enumerate(chunks)
                            if hv[h5][0] != "skip"] for h5 in range(2)]
                    ctxp = ps.tile([65, SN], f32, name=f"ctx{h}_{sn}",
                                   tag="psB", bufs=2, padded_shape=[128, 1024])
                    for ci, (t0, halves) in enumerate(chunks):
                        tci = t0 // 128
                        a0 = halves[0][0] != "skip"
                        a1 = halves[1][0] != "skip"
                        scp = ps.tile([128, SN], f32, name=f"sc{h}_{sn}_{tci}",
                                      tag="psA", bufs=2)
                        if a0:
                            nc.tensor.matmul(scp[:, 0:512],
                                             kbuf[:, t0:t0 + 128],
                                             qbuf[:, s0:s0 + 512],
                                             start=True, stop=True)
                        if a1:
                            nc.tensor.matmul(scp[:, 512:1024],
                                             kbuf[:, t0:t0 + 128],
                                             qbuf[:, s0 + 512:s0 + 1024],
                                             start=True, stop=True)
                        expt = sb.tile([128, SN], f32r, name=f"ex{h}_{sn}_{tci}",
                                       tag="expt", bufs=3)
                        if a0 and a1:
                            nc.scalar.activation(expt[:], scp[:], AF.Exp, scale=0.125)
                        elif a0:
                            nc.scalar.activation(expt[:, 0:512], scp[:, 0:512],
                                                 AF.Exp, scale=0.125)
                        else:
                            nc.scalar.activation(expt[:, 512:1024], scp[:, 512:1024],
                                                 AF.Exp, scale=0.125)
                        for h5, st in enumerate(halves):
                            base = h5 * 512
                            if st[0] == "tri":
                                off = st[1]
                                if off > 0:
                                    nc.vector.tensor_scalar_mul(
                                        expt[:, base:base + off],
                                        expt[:, base:base + off], 0.0)
                                nc.vector.tensor_mul(
                                    out=expt[:, base + off:base + off + 128],
                                    in0=expt[:, base + off:base + off + 128],
                                    in1=tri_r[:])
                            elif st[0] == "dram":
                                mst = sb.tile([128, 512], f32,
                                              name=f"mst{h}_{sn}_{tci}_{h5}",
                                              tag="mst", bufs=2)
                                nc.sync.dma_start(
                                    out=mst[:],
                                    in_=multT_d[t0:t0 + 128,
                                                s0 + base:s0 + base + 512])
                                mrr = sb.tile([128, 512], f32r,
                                              name=f"mrr{h}_{sn}_{tci}_{h5}",
                                              tag="mrr", bufs=2)
                                nc.vector.tensor_copy(mrr[:], mst[:])
                                nc.vector.tensor_mul(out=expt[:, base:base + 512],
                                                     in0=expt[:, base:base + 512],
                                                     in1=mrr[:])
                        for h5, a in enumerate((a0, a1)):
                            if not a:
                                continue
                            base = h5 * 512
                            nc.tensor.matmul(ctxp[:, base:base + 512],
                                             vbuf[:, tci * 65:(tci + 1) * 65],
                                             expt[:, base:base + 512],
                                             start=(ci == act[h5][0]),
                                             stop=(ci == act[h5][-1]))
                    rec = sb.tile([1, SN], f32, name=f"rec{h}_{sn}", tag="rec", bufs=1)
                    nc.vector.reciprocal(out=rec[:], in_=ctxp[64:65, :])
                    recb = sb.tile([64, SN], f32, name=f"recb{h}_{sn}",
                                   tag="recb", bufs=2)
                    nc.gpsimd.partition_broadcast(recb[:], rec[:])
                    mstg = sb.tile([64, SN], f32, name=f"mstg{h}_{sn}",
                                   tag="mstg", bufs=3)
                    nc.vector.tensor_mul(out=mstg[:], in0=ctxp[0:64, :], in1=recb[:])
                    nc.sync.dma_start(
                        out=cc_in[h * 64:(h + 1) * 64, s0:s0 + SN], in_=mstg[:])

            # ---- phase 3: AllGather contexts within batch group ---------
            nc.gpsimd.collective_compute(
                "AllGather",
                bacc.bass.mybir.AluOpType.bypass,
                replica_groups=GROUPS,
                ins=[cc_in.opt()],
                outs=[cc_out.opt()],
            )

            # ---- phase 4: output projection -----------------------------
            projp = [ps.tile([128, SN], f32, name=f"pj{i}",
                             tag=("psA" if i < 2 else "psB"), bufs=2)
                     for i in range(4)]   # i = oc*2 + sh2, [128, s-half]
            for k in range(KCH):
                wst = sb.tile([128, OC], f32, name=f"wpst{k}", tag="wst", bufs=2)
                nc.sync.dma_start(out=wst[:], in_=wproj_d[k * 128:(k + 1) * 128, :])
                wpr = sb.tile([128, OC], f32r, name=f"wpr{k}", tag="wv", bufs=KCH)
                nc.vector.tensor_copy(wpr[:], wst[:])
                for sh2 in range(2):
                    st = sb.tile([128, S // 2], f32, name=f"mgst{k}_{sh2}",
                                 tag="mgst", bufs=2)
                    nc.sync.dma_start(
                        out=st[:], in_=cc_out[k * 128:(k + 1) * 128,
                                              sh2 * 1024:(sh2 + 1) * 1024])
                    mr = sb.tile([128, S // 2], f32r, name=f"mgr{k}_{sh2}",
                                 tag="mgr", bufs=2)
                    nc.vector.tensor_copy(mr[:], st[:])
                    for oc in range(2):
                        for s4 in range(2):
                            nc.tensor.matmul(
                                projp[oc * 2 + sh2][:, s4 * 512:(s4 + 1) * 512],
                                wpr[:, oc * 128:(oc + 1) * 128],
                                mr[:, s4 * 512:(s4 + 1) * 512],
                                start=(k == 0), stop=(k == KCH - 1))
            for oc in range(2):
                for s4 in range(4):
                    pj = projp[oc * 2 + s4 // 2][:, (s4 % 2) * 512:(s4 % 2 + 1) * 512]
                    ost = sb.tile([128, 512], f32, name=f"ost{oc}_{s4}",
                                  tag="ost", bufs=3)
                    if has_bias:
                        nc.vector.tensor_scalar_add(ost[:], pj,
                                                    bpr_s[:, oc:oc + 1])
                    else:
                        nc.scalar.copy(out=ost[:], in_=pj)
                    nc.sync.dma_start(
                        out=outT_d[oc * 128:(oc + 1) * 128,
                                   s4 * 512:(s4 + 1) * 512], in_=ost[:])

    nc.finalize()
    return nc


def _numpy_fallback(x, mask, past_layer, w_attn, b_attn, w_proj, b_proj):
    qkv = np.einsum("bsd,de->bse", x, w_attn) + b_attn
    q, k, v = np.split(qkv, 3, axis=2)

    def sh(t):
        return t.reshape(B, S, H, DEPTH).transpose(0, 2, 1, 3)

    q, k, v = sh(q), sh(k), sh(v)
    k = np.concatenate([past_layer[:, 0], k], axis=2)
    v = np.concatenate([past_layer[:, 1], v], axis=2)
    present = np.stack([k, v], axis=1)
    scores = np.einsum("bhqd,bhkd->bhqk", q, k) / np.sqrt(np.float32(DEPTH))
    scores = scores + mask * np.float32(-1e9)
    scores = scores - scores.max(axis=-1, keepdims=True)
    e = np.exp(scores)
    attn = e / e.sum(axis=-1, keepdims=True)
    ctx = np.einsum("bhqk,bhkd->bhqd", attn, v)
    merged = ctx.transpose(0, 2, 1, 3).reshape(B, S, D)
    output = np.einsum("bsd,de->bse", merged, w_proj) + b_proj
    return output.astype(np.float32), present.astype(np.float32)


def kernel(x, mask, past_layer, w_attn, b_attn, w_proj, b_proj):
    global LAST_RESULT
    from concourse.bass_utils import run_bass_kernel_spmd

    x = np.asarray(x, dtype=np.float32)
    mask = np.asarray(mask, dtype=np.float32)
    past_layer = np.asarray(past_layer, dtype=np.float32)
    w_attn = np.asarray(w_attn, dtype=np.float32)
    b_attn = np.asarray(b_attn, dtype=np.float32)
    w_proj = np.asarray(w_proj, dtype=np.float32)
    b_proj = np.asarray(b_proj, dtype=np.float32)

    mask2d = np.ascontiguousarray(mask.reshape(S, T))
    mbool = mask2d != 0.0

    # degenerate fully-masked query rows diverge (reference softmax becomes
    # uniform); handle off-device
    if bool(mbool.all(axis=1).any()):
        return _numpy_fallback(x, mask, past_layer, w_attn, b_attn,
                               w_proj, b_proj)

    # canonical diagonal 128x128 pattern taken from the mask itself where
    # available, else the aligned-causal default
    tri_expect = np.tril(np.ones((128, 128), dtype=np.float32)).T
    diag = 1.0 - mbool[0:128, P:P + 128].T.astype(np.float32)
    if diag.min() == 0.0 and diag.max() == 1.0:
        tri_expect_c = diag
    else:
        tri_expect_c = tri_expect
    plan, mode = _build_plan(mbool, tri_expect_c)

    has_bias = bool(b_attn.any() or b_proj.any())
    key = _plan_key(plan, mode, has_bias)
    if key not in _prog_cache:
        _prog_cache[key] = _build_program(plan, mode, has_bias)
    nc = _prog_cache[key]

    # ---- host-side sharding prep ---------------------------------------
    xT = [np.ascontiguousarray(x[b].T) for b in range(B)]
    in_maps = []
    for c in range(N_CORES):
        b, g = c // 4, c % 4
        hs = list(range(4 * g, 4 * g + 4))
        qcols = np.concatenate([np.arange(64 * h, 64 * h + 64) for h in hs])
        kcols = qcols + D
        vcols = qcols + 2 * D
        m = {
            "xbT": xT[b],
            "wqk": np.ascontiguousarray(
                w_attn[:, np.concatenate([qcols, kcols])]),
            "wv": np.ascontiguousarray(w_attn[:, vcols]),
            "pastKT": np.ascontiguousarray(
                past_layer[b, 0, hs].transpose(0, 2, 1)),
            "pastV": np.ascontiguousarray(past_layer[b, 1, hs]),
            "tri": tri_expect_c,
            "wproj": np.ascontiguousarray(w_proj[:, OC * g:OC * (g + 1)]),
        }
        if has_bias:
            m["bqk"] = np.ascontiguousarray(
                b_attn[np.concatenate([qcols, kcols])])
            m["bv"] = np.ascontiguousarray(b_attn[vcols])
            m["bproj"] = np.ascontiguousarray(b_proj[OC * g:OC * (g + 1)])
        if mode == "general":
            m["multT"] = np.ascontiguousarray(
                (1.0 - mask2d).T.astype(np.float32))
        in_maps.append(m)

    res = run_bass_kernel_spmd(nc, in_maps, list(range(N_CORES)))
    LAST_RESULT = res

    # ---- unshard -------------------------------------------------------
    output = np.empty((B, S, D), dtype=np.float32)
    present = np.empty((B, 2, H, T, DEPTH), dtype=np.float32)
    present[:, 0, :, :P] = past_layer[:, 0]
    present[:, 1, :, :P] = past_layer[:, 1]
    for c in range(N_CORES):
        b, g = c // 4, c % 4
        r = res.results[c]
        output[b, :, OC * g:OC * (g + 1)] = r["outT"].T
        for i, h in enumerate(range(4 * g, 4 * g + 4)):
            present[b, 0, h, P:] = r["pkT"][i].T
            present[b, 1, h, P:] = r["pv"][i]
    return output, present
